# revision 9
# baseline (speedup 1.0000x reference)
"""MoE layer (top-2, E=8, capacity-dropped) on 8 TRN2 NeuronCores.

Strategy (final-expert dedup + balanced chunk-parallel):
  - The reference scatters expert outputs with plain writes in expert order,
    so later experts overwrite earlier ones: each token's output comes ONLY
    from the highest-indexed expert where it survives capacity. Instead of
    running all 8*3277 capacity slots (26k token-expert pairs), we compute
    each token once, under its final expert (~13.4k pairs on this input) —
    a ~2x compute cut.
  - Router runs on host via jax CPU, mirroring the reference ops exactly
    (bit-for-bit top-2 + capacity cutoffs). Router flops are ~0.06% of total.
  - Work distribution: the per-expert final-token counts are heavily skewed
    (expert 7 keeps everything it saw, expert 0 almost nothing), so experts
    are NOT pinned to cores. Each core runs the same program: 6 token chunks
    of sizes (512,256,128,512,256,128) = 1792 slots, each chunk carrying its
    own expert weights streamed from DRAM. The host greedily bin-packs
    (expert, token-slice) segments onto the 48 global chunk slots, so every
    core gets exactly 1792 slots of MLP work (vs 3328 for naive
    expert-per-core).
  - Math in bfloat16 on the PE (same 1 cycle/row as fp32r, half the HBM
    traffic; fp32 PSUM accumulation; ~2e-3 rel err vs the fp32 reference).
    Weights are pre-shuffled on host into [128, 4096]-contiguous DMA blocks
    so each chunk streams w1+w2 (16MB bf16) in 16 large full-bandwidth DMAs.
    All DMAs ride one queue (SP) so the global DMA-engine FIFO delivers in
    consumption order — mixing queues let prefetches jump the line and
    stalled the PE.
  - Per chunk: layer1 keeps x^T stationary (psum[f,tok] over 8 d-tiles,
    gelu+b1 fused on ScalarE into bf16 H^T), layer2 streams w2 (psum[tok,col]
    over 32 f-tiles), b2 added on the PSUM->SBUF move (VectorE), fp32 out.
    Chunk 0 ramps the PE early via quarter-granularity w1/x interleaved DMAs
    with k-pair psum passes; the last chunk's final half-column psums are
    split so the drain pipeline is shorter. Cost model: ~393 us/core at ~97%
    PE occupancy (PE floor 382 us).
  - Host combine: scatter each chunk's rows back to its token slice;
    tokens dropped by all experts stay zero.
"""

import numpy as np

B, S, D, DFF, E, TOPK = 8, 2048, 1024, 4096, 8, 2
T = B * S                 # 16384 tokens
CAP = 3277                # ceil(T * 1.6 / 8)
NOISE_STD = 0.02
N_CORES = 8
# Per-core chunk sizes (multiples of 128). Order interleaves small chunks
# after big ones so the per-chunk weight-stream deficit of small chunks is
# covered by the DMA lead built during 512-token chunks.
CHUNKS = (512, 256, 128, 512, 256, 128)   # 1792 slots/core, 14336 total

_CACHE = {}


def _build_nc(chunks, w1_bufs=5, w2_bufs=11, xt_bufs=2, ot_bufs=5,
              ps1_bufs=4, tail_pieces=2):
    import concourse.mybir as mybir
    import concourse.tile as tile
    from concourse import bacc

    DT = mybir.dt.float32
    BF = mybir.dt.bfloat16
    GELU = mybir.ActivationFunctionType.Gelu

    nc = bacc.Bacc("TRN2", target_bir_lowering=False, debug=False,
                   num_devices=N_CORES)
    xT_d, w1_d, w2_d, b1_d, b2_d, out_d = [], [], [], [], [], []
    for c, csz in enumerate(chunks):
        xT_d.append(nc.dram_tensor(f"xT_{c}", [128, 8 * csz], BF,
                                   kind="ExternalInput").ap())
        # host-shuffled: row g*128+p, col k*512+f  (g = f-group of 512)
        w1_d.append(nc.dram_tensor(f"w1_{c}", [1024, 4096], BF,
                                   kind="ExternalInput").ap())
        # host-shuffled: row (n*4+gg)*128+p, col gi*512+c (f=(gg*8+gi)*128+p)
        w2_d.append(nc.dram_tensor(f"w2_{c}", [1024, 4096], BF,
                                   kind="ExternalInput").ap())
        b1_d.append(nc.dram_tensor(f"b1_{c}", [128, 32], DT,
                                   kind="ExternalInput").ap())
        b2_d.append(nc.dram_tensor(f"b2_{c}", [128, 1024], DT,
                                   kind="ExternalInput").ap())
        out_d.append(nc.dram_tensor(f"out_{c}", [csz, 1024], DT,
                                    kind="ExternalOutput").ap())

    with tile.TileContext(nc) as tc:
        with (
            tc.tile_pool(name="xt", bufs=xt_bufs) as xt_pool,
            tc.tile_pool(name="ht", bufs=1) as ht_pool,
            tc.tile_pool(name="w1p", bufs=w1_bufs) as w1_pool,
            tc.tile_pool(name="w2p", bufs=w2_bufs) as w2_pool,
            tc.tile_pool(name="b1p", bufs=2) as b1_pool,
            tc.tile_pool(name="b2p", bufs=2) as b2_pool,
            tc.tile_pool(name="outp", bufs=ot_bufs) as out_pool,
            tc.tile_pool(name="ps1", bufs=ps1_bufs, space="PSUM") as ps1_pool,
            tc.tile_pool(name="ps2", bufs=1, space="PSUM") as ps2_pool,
        ):
            for c, csz in enumerate(chunks):
                ntt = csz // 128
                last = (c == len(chunks) - 1)
                xt = xt_pool.tile([128, 8 * csz], BF, tag="xt", name="xt")
                ht = ht_pool.tile([128, 32 * csz], BF, tag="ht", name="ht")
                b1_sb = b2_sb = None

                # ---- layer 1: H^T[f, tok] = gelu(sum_k w1_kf.T @ xt_k + b1)
                for g in range(8):
                    w1g = w1_pool.tile([128, 4096], BF, tag="w1g", name="w1g")
                    if g == 0 and c == 0:
                        # startup: quarter-granularity interleave of w1/x so
                        # the PE starts after ~1.5us of DMA instead of ~6us
                        for q in range(4):
                            nc.sync.dma_start(
                                w1g[:, q * 1024:(q + 1) * 1024],
                                w1_d[c][0:128, q * 1024:(q + 1) * 1024])
                            nc.sync.dma_start(
                                xt[:, q * 2 * csz:(q + 1) * 2 * csz],
                                xT_d[c][:, q * 2 * csz:(q + 1) * 2 * csz])
                    elif g == 0:
                        nc.sync.dma_start(w1g[:],
                                          w1_d[c][g * 128:(g + 1) * 128, :])
                        nc.sync.dma_start(xt[:, 0:4 * csz],
                                          xT_d[c][:, 0:4 * csz])
                        nc.sync.dma_start(xt[:, 4 * csz:8 * csz],
                                          xT_d[c][:, 4 * csz:8 * csz])
                    else:
                        nc.sync.dma_start(w1g[:],
                                          w1_d[c][g * 128:(g + 1) * 128, :])
                    if g == 0:
                        b1_sb = b1_pool.tile([128, 32], DT, tag="b1",
                                             name="b1")
                        nc.sync.dma_start(b1_sb[:], b1_d[c][:, :])
                        b2_sb = b2_pool.tile([128, 1024], DT, tag="b2",
                                             name="b2")
                        nc.sync.dma_start(b2_sb[:], b2_d[c][:, :])

                    if g == 0 and c == 0:
                        # k-pair psum passes matching the quarter DMAs
                        pss1 = [ps1_pool.tile([128, csz], DT, tag="ps1",
                                              name="ps1") for _ in range(4)]
                        for kp in range(4):
                            for fi in range(4):
                                for k in (2 * kp, 2 * kp + 1):
                                    nc.tensor.matmul(
                                        pss1[fi][:],
                                        lhsT=w1g[:, k * 512 + fi * 128:
                                                 k * 512 + (fi + 1) * 128],
                                        rhs=xt[:, k * csz:(k + 1) * csz],
                                        start=(k == 0), stop=(k == 7))
                        for fi in range(4):
                            nc.scalar.activation(
                                ht[:, fi * csz:(fi + 1) * csz], pss1[fi][:],
                                GELU, bias=b1_sb[:, fi:fi + 1])
                    else:
                        for fi in range(4):
                            f_t = g * 4 + fi
                            ps = ps1_pool.tile([128, csz], DT, tag="ps1",
                                               name="ps1")
                            for k in range(8):
                                nc.tensor.matmul(
                                    ps[:],
                                    lhsT=w1g[:, k * 512 + fi * 128:
                                             k * 512 + (fi + 1) * 128],
                                    rhs=xt[:, k * csz:(k + 1) * csz],
                                    start=(k == 0), stop=(k == 7))
                            nc.scalar.activation(
                                ht[:, f_t * csz:(f_t + 1) * csz], ps[:],
                                GELU, bias=b1_sb[:, f_t:f_t + 1])

                # ---- layer 2: out[tok, :] = H^T.T @ w2 + b2
                for n in range(2):
                    npieces = tail_pieces if (last and n == 1) else 1
                    psz = 512 // npieces
                    pss = [[ps2_pool.tile([128, psz], DT,
                                          tag=f"ps2_{t * npieces + pc}",
                                          name=f"ps2_{t * npieces + pc}")
                            for pc in range(npieces)] for t in range(ntt)]
                    for gg in range(4):
                        w2g = w2_pool.tile([128, 4096], BF, tag="w2g",
                                           name="w2g")
                        nc.sync.dma_start(
                            w2g[:], w2_d[c][(n * 4 + gg) * 128:
                                            (n * 4 + gg + 1) * 128, :])
                        for gi in range(8):
                            f_t = gg * 8 + gi
                            for t in range(ntt):
                                for pc in range(npieces):
                                    nc.tensor.matmul(
                                        pss[t][pc][:],
                                        lhsT=ht[:, f_t * csz + t * 128:
                                                f_t * csz + t * 128 + 128],
                                        rhs=w2g[:, gi * 512 + pc * psz:
                                                gi * 512 + (pc + 1) * psz],
                                        start=(f_t == 0), stop=(f_t == 31))
                    for t in range(ntt):
                        ot = out_pool.tile([128, 512], DT, tag="ot",
                                           name="ot")
                        for pc in range(npieces):
                            sl = slice(pc * psz, (pc + 1) * psz)
                            nc.vector.tensor_add(
                                ot[:, sl], pss[t][pc][:],
                                b2_sb[:, n * 512 + pc * psz:
                                      n * 512 + (pc + 1) * psz])
                            nc.sync.dma_start(
                                out_d[c][t * 128:(t + 1) * 128,
                                         n * 512 + pc * psz:
                                         n * 512 + (pc + 1) * psz],
                                ot[:, sl])
    nc.compile()
    return nc


def _get_nc():
    key = CHUNKS
    if key not in _CACHE:
        _CACHE[key] = _build_nc(CHUNKS)
    return _CACHE[key]


def _route(x_flat, noise, router_w, router_b):
    """Mirror of the reference router, on jax CPU (decisions verified to
    match the reference backend bit-for-bit on this input distribution)."""
    import jax
    import jax.numpy as jnp

    cpu = jax.devices("cpu")[0]
    with jax.default_device(cpu):
        xj = jnp.asarray(x_flat)
        logits = (xj @ jnp.asarray(router_w).T + jnp.asarray(router_b)
                  + jnp.asarray(noise) * NOISE_STD)
        probs = jax.nn.softmax(logits, axis=-1)
        _, topk_idx = jax.lax.top_k(probs, TOPK)
    return np.asarray(topk_idx)


def _pack(counts):
    """Greedy bin-pack: cover each expert's token count with whole chunk
    slots from the global pool (8 cores x CHUNKS). Returns
    (assignments, leftovers) where assignments are
    (expert, tok_offset, core, chunk_idx, take) and leftovers are
    (expert, tok_offset, n_left) for tokens that did not fit (never happens
    for counts summing <= 13352; handled on host as a safety net)."""
    avail = {}
    for core in range(N_CORES):
        for ci, sz in enumerate(CHUNKS):
            avail.setdefault(sz, []).append((core, ci))
    sizes_desc = sorted(avail, reverse=True)
    assign = []
    leftovers = []
    for e in np.argsort(counts)[::-1]:
        need = int(counts[e])
        off = 0
        while need > 0:
            pick = None
            for s in sizes_desc:          # largest chunk fully used by need
                if avail[s] and s <= need:
                    pick = s
                    break
            if pick is None:              # smallest chunk covering the tail
                cands = [s for s in sizes_desc if avail[s]]
                if not cands:
                    leftovers.append((int(e), off, need))
                    break
                pick = min(cands)
            core, ci = avail[pick].pop()
            take = min(need, pick)
            assign.append((int(e), off, core, ci, take))
            off += take
            need -= take
    return assign, leftovers


def kernel(x, noise, router_w, router_b, w1, b1, w2, b2):
    import ml_dtypes
    from concourse.bass_utils import run_bass_kernel_spmd

    BF = ml_dtypes.bfloat16
    x = np.asarray(x, dtype=np.float32)
    noise = np.asarray(noise, dtype=np.float32)
    router_w = np.asarray(router_w, dtype=np.float32)
    router_b = np.asarray(router_b, dtype=np.float32)
    w1 = np.asarray(w1, dtype=np.float32)
    b1 = np.asarray(b1, dtype=np.float32)
    w2 = np.asarray(w2, dtype=np.float32)
    b2 = np.asarray(b2, dtype=np.float32)

    x_flat = x.reshape(T, D)
    topk_idx = _route(x_flat, noise, router_w, router_b)

    # Final owner of each token: the highest expert where it survives
    # capacity (reference writes in expert order; later writes win).
    final = np.full(T, -1, np.int64)
    for e in range(E):
        nz = np.flatnonzero((topk_idx == e).any(axis=-1))[:CAP]
        final[nz] = e
    toks_of = [np.flatnonzero(final == e) for e in range(E)]
    counts = np.array([len(t) for t in toks_of])
    assign, leftovers = _pack(counts)

    # Pre-shuffled bf16 weights, one per expert (shared across chunks).
    # w1 tile layout: row g*128+p, col k*512+f  <- w1[e][k*128+p, g*512+f]
    # w2 tile layout: row (n*4+gg)*128+p, col gi*512+c
    #                 <- w2[e][(gg*8+gi)*128+p, n*512+c]
    w1bf = w1.astype(BF)
    w2bf = w2.astype(BF)
    w1t = [np.ascontiguousarray(
        w1bf[e].reshape(8, 128, 8, 512).transpose(2, 1, 0, 3)
        ).reshape(1024, 4096) for e in range(E)]
    w2t = [np.ascontiguousarray(
        w2bf[e].reshape(4, 8, 128, 2, 512).transpose(3, 0, 2, 1, 4)
        ).reshape(1024, 4096) for e in range(E)]
    b1t = [np.ascontiguousarray(b1[e].reshape(32, 128).T) for e in range(E)]
    b2t = [np.ascontiguousarray(
        np.broadcast_to(b2[e], (128, 1024))) for e in range(E)]
    xTbf = np.ascontiguousarray(x_flat.astype(BF).reshape(T, 8, 128)
                                .transpose(2, 1, 0))   # [128, 8, T]

    zw = np.zeros((1024, 4096), BF)
    zb1 = np.zeros((128, 32), np.float32)
    zb2 = np.zeros((128, 1024), np.float32)
    in_maps = [{} for _ in range(N_CORES)]
    for core in range(N_CORES):
        for ci, csz in enumerate(CHUNKS):
            in_maps[core][f"xT_{ci}"] = np.zeros((128, 8 * csz), BF)
            in_maps[core][f"w1_{ci}"] = zw
            in_maps[core][f"w2_{ci}"] = zw
            in_maps[core][f"b1_{ci}"] = zb1
            in_maps[core][f"b2_{ci}"] = zb2
    for e, off, core, ci, take in assign:
        csz = CHUNKS[ci]
        toks = toks_of[e][off:off + take]
        xTa = np.zeros((128, 8, csz), BF)
        xTa[:, :, :take] = xTbf[:, :, toks]
        in_maps[core][f"xT_{ci}"] = xTa.reshape(128, 8 * csz)
        in_maps[core][f"w1_{ci}"] = w1t[e]
        in_maps[core][f"w2_{ci}"] = w2t[e]
        in_maps[core][f"b1_{ci}"] = b1t[e]
        in_maps[core][f"b2_{ci}"] = b2t[e]

    nc = _get_nc()
    res = None
    last_exc = None
    for attempt in range(3):
        try:
            res = run_bass_kernel_spmd(nc, in_maps,
                                       core_ids=list(range(N_CORES)))
            break
        except Exception as exc:   # transient axon/device hiccups recover
            last_exc = exc
            import time
            time.sleep(5.0 * (attempt + 1))
    if res is None:
        raise last_exc

    out_flat = np.zeros((T, D), dtype=np.float32)
    for e, off, core, ci, take in assign:
        toks = toks_of[e][off:off + take]
        out_flat[toks] = res.results[core][f"out_{ci}"][:take]

    if leftovers:   # safety net, unreachable for this input distribution
        import jax
        import jax.numpy as jnp
        cpu = jax.devices("cpu")[0]
        with jax.default_device(cpu):
            for e, off, n_left in leftovers:
                toks = toks_of[e][off:off + n_left]
                h = jax.nn.gelu(jnp.asarray(x_flat[toks]) @ jnp.asarray(w1[e])
                                + jnp.asarray(b1[e]), approximate=False)
                eo = h @ jnp.asarray(w2[e]) + jnp.asarray(b2[e])
                out_flat[toks] = np.asarray(eo)
    return out_flat.reshape(B, S, D)
